# revision 39
# baseline (speedup 1.0000x reference)
"""ChunkedAttention (nn_ChunkedAttention_43568148251092) Trainium2 kernel.

Full inputs q/k/v: [1, 4096, 16, 128] fp32. Shards the 16 heads across the
8 NeuronCores (2 heads per core, pure head parallelism - no collectives),
runs a Bass/Tile attention kernel per core, and concatenates the results.

Per-head pipeline on each core (S=4096 tokens, D=128):
  - int8 quant-dequant of K and V per token, trunc-toward-zero exactly as the
    reference. Trunc via sign-offset + RNE int convert: t = (x>0)*0.9998;
    i = rne_i32(x + 0.4999 - t). Exact for integer x (incl +/-127).
    Kint kept as fp16 integers (exact: |int| <= 127); per-token kscale kept
    fp32 and folded into the softmax exp via the ACT per-partition scale.
    V dequantized to bf16 (Vint * vscale).
  - Q cast to fp16; Q and Kint transposed to [d, s] via PE transpose.
  - S^T[k,q] = Kint^T.T @ Q^T in PSUM fp32 (single 128-deep matmul).
  - P'[k,q] = exp(kscale/sqrt(D) * S^T - 40): most key tiles via ScalarE
    (table exp); every 6th tile (every 4th in the final ACT-paced window,
    dve_mod2) via a DVE Schraudolph fast-exp --
    bits_i16 = rne(s*ksl*128*log2e + (128*(127-40*log2e) - 5.5)) written
    through an int16-bitcast AP and reused as bf16 (mantissa-linear 2^x
    interpolation; the +-3% sawtooth is zero-mean after the -5.5 magic and
    softmax-normalizes out; measured end-to-end rel-err 7.6e-3 vs 2e-2 gate).
  - out[q, 0:128|denom] = sum_kt P'_kt.T @ [Vdq | ones] accumulated in PSUM;
    the ones-column yields the softmax denominator for free.
  - out normalized via psO->SBUF copy (DVE), batched reciprocal, and a
    GpSimd multiply; DMA to DRAM.

Why the exp split: ACT alone is the bottleneck (0.833 ns/elem + 185 ns fixed
per instruction = 273 us busy for 2 heads); PE busy is 226.5 us (bf16
compute roofline: QK 109 us + PV 110 us + transposes 7 us). Offloading
5/32 key tiles per (head,qc) window to DVE brings ACT to ~228 us so the
steady state is PE-bound at ~98-100% PE occupancy.

Scheduling structure (what makes the static schedule robust):
  - DVE exp tiles get their own single-bank PSUM pool (psD, [128,512] x2
    exps per tile) so DVE latency jitter never gates the ACT/psS pipeline;
    PSUM banks: psS 2x2 + psD 1 + psT 1 + psO 2x1 = 8.
  - The first two windows (head 0, qc0 with qc1 ramping via dve_late) run
    all-ACT: DVE is saturated there with head-0 K/V quant chains (the
    engines execute their streams in order, so a long-wait instruction at a
    queue head stalls everything behind it - keep DVE self-paced).
  - PV normalization releases psO via a fast DVE copy; the divide runs
    batched (4 q-tiles) on GpSimd off the critical path; the final groups
    normalize on DVE (GpSimd launch latency would sit on the drain tail).
  - All quant chains stay on DVE: GpSimd (Pool) chain segments always lost
    in sim (slow per-op + cross-engine sem latency poisons chain deadlines).
    Pool does only Q fp32->fp16 casts, Vext ones-memsets, PV normalize.
    Pool ISA notes: no scalar_tensor_tensor, no tensor_reduce, no
    mixed-dtype integer TensorTensor (see the eng==Pool branches).
  - Head h+1 K/Q chains hook into head h's qc1/qc2 kt slots (+2 offset to
    dodge PV-group boundaries); head-0 V chains hook into (0,0) slots.
  - output stores batched 512 tokens per DMA, except the closing drain,
    which normalizes and stores per 128-token tile so earlier tiles stream
    out while PE finishes the last PV groups.

TimelineSim: 264,572 ns (baseline this was forked from: 292,855 ns).
"""

import functools
import math
import time

import numpy as np

import concourse.bass as bass
import concourse.mybir as mybir
import concourse.tile as tile
from concourse import bacc
from concourse.bass_utils import run_bass_kernel_spmd
from concourse.masks import make_identity

F32 = mybir.dt.float32
BF16 = mybir.dt.bfloat16
FP16 = mybir.dt.float16
I16 = mybir.dt.int16
I32 = mybir.dt.int32
AX = mybir.AxisListType.X
OP = mybir.AluOpType
EXP = mybir.ActivationFunctionType.Exp

# Schraudolph fast-exp on DVE: bf16(bits) ~= exp(x) for
# bits = rne(x*128*log2e + 128*127 + magic). With the exp bias -40 folded in
# and the per-key scale ksl applied to the raw score s:
#   bits = s * (ksl*128*log2e) + (128*(127 - 40*log2e) - 5.5)
# -5.5 centers the mantissa-linear-interp error (+-3% relative on P, which
# softmax-normalizes to ~0 mean); measured end-to-end rel-err contribution
# ~1e-3 per offloaded tile fraction of 1/32.
_LOG2E = 1.4426950408889634
_SCHRAUD_A = 128.0 * _LOG2E          # multiplies ksl
_SCHRAUD_B = 128.0 * (127.0 - 40.0 * _LOG2E) - 5.5

_S = 4096
_H_TOTAL = 16
_D = 128
_N_CORES = 8
_H = _H_TOTAL // _N_CORES  # heads per core

_NC_CACHE = {}


def _bcast3(ap2, n):
    """[128, J] AP -> [128, J, n] broadcast AP (inner stride 0)."""
    return bass.AP(tensor=ap2.tensor, offset=ap2.offset, ap=[*ap2.ap, [0, n]])


def _build_nc(S=_S, H=_H, D=_D, qk_dt=FP16, pp_bufs=60, ld_bufs=9,
              psS_bufs=2, psT_bufs=1, psO_bufs=2, pre_emit="qc0",
              pv_chunk=4, cast_eng="gpsimd", ones_eng="gpsimd",
              split_first=True, v_pool=0, kq_pos=None, v_pos=None,
              dve_mod=4, dve_off=3, dve_skip=(), dve_cols=0,
              chain_eng="gpsimd"):
    if dve_late is not None and not isinstance(dve_late, dict):
        dve_late = {tuple(dve_late[0]): dve_late[1]}
    assert D == 128 and S % 512 == 0
    n_kt = S // 128          # 32 key tiles of 128 tokens
    n_ch = S // 512          # 8 chunks of 512 tokens
    n_qc = S // 1024         # 4 query column blocks of 1024

    nc = bacc.Bacc("TRN2")
    q_d = nc.dram_tensor("q", [S, H, D], F32, kind="ExternalInput")
    k_d = nc.dram_tensor("k", [S, H, D], F32, kind="ExternalInput")
    v_d = nc.dram_tensor("v", [S, H, D], F32, kind="ExternalInput")
    o_d = nc.dram_tensor("o", [S, H, D], F32, kind="ExternalOutput")

    with tile.TileContext(nc) as tc:
        with (
            tc.tile_pool(name="const", bufs=1) as constp,
            tc.tile_pool(name="big", bufs=2) as bigp,
            tc.tile_pool(name="ld", bufs=ld_bufs) as ldp,
            tc.tile_pool(name="tmp", bufs=2) as tmpp,
            tc.tile_pool(name="b16", bufs=2) as b16p,
            tc.tile_pool(name="small", bufs=smallp_bufs) as smallp,
            tc.tile_pool(name="pp", bufs=pp_bufs) as ppool,
            tc.tile_pool(name="outp", bufs=outp_bufs) as outp,
            tc.tile_pool(name="psT", bufs=psT_bufs, space="PSUM") as psT,
            tc.tile_pool(name="psS", bufs=psS_bufs, space="PSUM") as psS,
            tc.tile_pool(name="psO", bufs=psO_bufs, space="PSUM") as psO,
            tc.tile_pool(name="psD", bufs=1, space="PSUM") as psD,
        ):
            bias_t = constp.tile([128, 1], F32)
            nc.vector.memset(bias_t[:], -40.0)
            # Dummy activation emitted first so the ACT table load happens
            # at t~0 instead of just before the first real exp.
            warm = constp.tile([128, 1], F32)
            nc.scalar.activation(warm[:], bias_t[:], EXP, bias=bias_t[:])
            ident32 = constp.tile([128, 128], F32)
            make_identity(nc, ident32[:])
            ident16 = constp.tile([128, 128], qk_dt)
            nc.vector.tensor_copy(ident16[:], ident32[:])
            ceng = getattr(nc, cast_eng)
            oeng = getattr(nc, ones_eng)
            cheng = getattr(nc, chain_eng)

            def emit_k(h, hd, c, j0=0, nj=4, dest=None, ksdest=None,
                       ksadest=None, eng=None):
                """Quantize K tokens [c*512+j0*128, +nj*128) of head h into
                dest (default: the chunk's KT tile) + kscale tile."""
                s0 = c * 512 + j0 * 128
                kf = ldp.tile([128, nj, 128], F32, tag="ld", name="kf")
                nc.sync.dma_start(
                    out=kf[:],
                    in_=k_d[s0:s0 + nj * 128, h, :].rearrange(
                        "(j p) d -> p j d", p=128))
                am = smallp.tile([128, nj], F32, tag="am", name="am")
                nc.vector.reduce_max(am[:], kf[:], axis=AX,
                                     apply_absolute_value=True)
                sc = smallp.tile([128, nj], F32, tag="sc", name="sc")
                nc.vector.tensor_scalar(sc[:], am[:], 1e-8, 1.0 / 127.0,
                                        op0=OP.max, op1=OP.mult)
                ks = ksdest if ksdest is not None else hd["ks"][c]
                nc.vector.tensor_scalar(ks[:], sc[:], 1.0 / math.sqrt(128.0),
                                        None, op0=OP.mult)
                ksa = ksadest if ksadest is not None else hd["ksA"][c]
                nc.vector.tensor_scalar(ksa[:], sc[:],
                                        _SCHRAUD_A / math.sqrt(128.0),
                                        None, op0=OP.mult)
                rc = smallp.tile([128, nj], F32, tag="rc", name="rc")
                nc.vector.reciprocal(rc[:], sc[:])
                ke = cheng
                ke.tensor_tensor(kf[:], kf[:], _bcast3(rc[:], 128),
                                 op=OP.mult)  # x, in-place
                t = tmpp.tile([128, nj, 128], F32, tag="t", name="t")
                ke.tensor_scalar(t[:], kf[:], 0.0, 0.9998,
                                 op0=OP.is_gt, op1=OP.mult)
                i32 = tmpp.tile([128, nj, 128], I32, tag="i32", name="i32")
                if ke.engine == mybir.EngineType.Pool:
                    # Pool ISA: no scalar_tensor_tensor, and TensorTensor
                    # with int out requires matching dtypes. Compute the
                    # pre-round value in f32 (in place), then RNE via copy.
                    ke.tensor_scalar(kf[:], kf[:], 0.4999, None, op0=OP.add)
                    ke.tensor_tensor(t[:], kf[:], t[:], op=OP.subtract)
                    ke.tensor_copy(i32[:], t[:])
                else:
                    ke.scalar_tensor_tensor(i32[:], kf[:], 0.4999, t[:],
                                            op0=OP.add, op1=OP.subtract)
                k16 = b16p.tile([128, nj, 128], qk_dt, tag="k16", name="k16")
                ke.tensor_copy(k16[:], i32[:])
                pst = psT.tile([128, 4, 128], qk_dt, tag="pst", name="pst")
                for j in range(nj):
                    nc.tensor.transpose(pst[:, j, :], k16[:, j, :], ident16[:])
                if dest is None:
                    dest = hd["KT"][c]
                nc.vector.tensor_copy(dest[:], pst[:, 0:nj, :])

            def emit_q(h, hd, c):
                """Cast+transpose Q for chunk c (512 tokens) of head h."""
                s0 = c * 512
                qf = ldp.tile([128, 4, 128], F32, tag="ld")
                nc.sync.dma_start(
                    out=qf[:],
                    in_=q_d[s0:s0 + 512, h, :].rearrange(
                        "(j p) d -> p j d", p=128))
                q16 = b16p.tile([128, 4, 128], qk_dt, tag="q16")
                ceng.tensor_copy(q16[:], qf[:])
                pst2 = psT.tile([128, 4, 128], qk_dt, tag="pst")
                for j in range(4):
                    nc.tensor.transpose(pst2[:, j, :], q16[:, j, :],
                                        ident16[:])
                nc.vector.tensor_copy(hd["QT"][c][:], pst2[:])

            def emit_kq(h, hd, c):
                emit_k(h, hd, c)
                emit_q(h, hd, c)

            def emit_v(h, hd, c, eng=None):
                """Quantize-dequantize V chunk c of head h into Vext."""
                e = eng if eng is not None else cheng
                s0 = c * 512
                vf = ldp.tile([128, 4, 128], F32, tag="ld", name="vf")
                nc.sync.dma_start(
                    out=vf[:],
                    in_=v_d[s0:s0 + 512, h, :].rearrange(
                        "(j p) d -> p j d", p=128))
                am2 = smallp.tile([128, 4], F32, tag="am", name="am2")
                # free-axis reduce is DVE-only; cheap (1 of 6 passes)
                nc.vector.reduce_max(am2[:], vf[:], axis=AX,
                                     apply_absolute_value=True)
                sc2 = smallp.tile([128, 4], F32, tag="sc", name="sc2")
                e.tensor_scalar(sc2[:], am2[:], 1e-8, 1.0 / 127.0,
                                op0=OP.max, op1=OP.mult)
                rc2 = smallp.tile([128, 4], F32, tag="rc", name="rc2")
                nc.vector.reciprocal(rc2[:], sc2[:])  # DVE-only op (tiny)
                e.tensor_tensor(vf[:], vf[:], _bcast3(rc2[:], 128),
                                op=OP.mult)
                t2 = tmpp.tile([128, 4, 128], F32, tag="t", name="t2")
                e.tensor_scalar(t2[:], vf[:], 0.0, 0.9998,
                                op0=OP.is_gt, op1=OP.mult)
                i32v = tmpp.tile([128, 4, 128], I32, tag="i32", name="i32v")
                if e.engine == mybir.EngineType.Pool:
                    e.tensor_scalar(vf[:], vf[:], 0.4999, None, op0=OP.add)
                    e.tensor_tensor(t2[:], vf[:], t2[:], op=OP.subtract)
                    e.tensor_copy(i32v[:], t2[:])
                    # mixed-dtype (i32 x f32 -> bf16) TensorTensor is not
                    # legal on Pool; the dequant multiply goes to DVE.
                    ve = nc.vector
                else:
                    e.scalar_tensor_tensor(i32v[:], vf[:], 0.4999, t2[:],
                                           op0=OP.add, op1=OP.subtract)
                    ve = e
                vext = hd["V"]
                ve.tensor_tensor(
                    vext[:, 4 * c:4 * c + 4, 0:128], i32v[:],
                    _bcast3(sc2[:], 128), op=OP.mult)
                oeng.memset(vext[:, 4 * c:4 * c + 4, 128:129], 1.0)

            def make_hd(h, split_first=False):
                hd = {
                    "KT": [bigp.tile([128, 512], qk_dt, tag=f"KT{c}",
                                     name=f"KT{c}") for c in range(n_ch)],
                    "QT": [bigp.tile([128, 512], qk_dt, tag=f"QT{c}",
                                     name=f"QT{c}") for c in range(n_ch)],
                    "V": bigp.tile([128, n_kt, 129], BF16, tag="V", name="V"),
                    "ks": [bigp.tile([128, 4], F32, tag=f"ks{c}",
                                     name=f"ks{c}") for c in range(n_ch)],
                    "ksA": [bigp.tile([128, 4], F32, tag=f"ksA{c}",
                                      name=f"ksA{c}") for c in range(n_ch)],
                    "h": h,
                }
                if split_first:
                    # Halved first-chunk tiles so the very first exp is not
                    # gated on the full 512-token K chain (fill latency).
                    hd["KT0h"] = [bigp.tile([128, 256], qk_dt, tag=f"KT0{i}",
                                            name=f"KT0{i}") for i in range(2)]
                    hd["ks0h"] = [bigp.tile([128, 2], F32, tag=f"ks0{i}",
                                            name=f"ks0{i}") for i in range(2)]
                    hd["ksA0h"] = [bigp.tile([128, 2], F32, tag=f"ksA0{i}",
                                             name=f"ksA0{i}") for i in range(2)]
                return hd

            def kt_weight(hd, kt):
                """[128(d), 128(k)] stationary slice + kscale/ksA [128,1]."""
                if kt < 4 and "KT0h" in hd:
                    tl = hd["KT0h"][kt // 2]
                    ks = hd["ks0h"][kt // 2]
                    ksa = hd["ksA0h"][kt // 2]
                    return (tl[:, (kt % 2) * 128:(kt % 2 + 1) * 128],
                            ks[:, kt % 2:kt % 2 + 1],
                            ksa[:, kt % 2:kt % 2 + 1])
                return (hd["KT"][kt // 4][:, (kt % 4) * 128:(kt % 4 + 1) * 128],
                        hd["ks"][kt // 4][:, kt % 4:kt % 4 + 1],
                        hd["ksA"][kt // 4][:, kt % 4:kt % 4 + 1])

            def emit_pv(prev, j):
                """Attention-weighted V for query tile j (128 q) of a
                completed (head, qbase, npv*128 cols) score block. The psO
                accumulator is released by a fast DVE copy; the denominator
                normalize runs batched (4 q tiles) on GpSimd so DVE's
                in-order stream never gates PSUM reuse on it."""
                pts, vext, h, qbase, _ = prev
                ops_ = psO.tile([128, 129], F32, tag="ops")
                for kt in range(n_kt):
                    nc.tensor.matmul(
                        ops_[:], pts[kt][:, j * 128:(j + 1) * 128],
                        vext[:, kt, 0:129],
                        start=(kt == 0), stop=(kt == n_kt - 1))
                if j % 4 == 0:
                    prev_ot[0] = outp.tile([128, 4, 129], F32, tag="ot",
                                           name="ot")
                ot = prev_ot[0]
                nc.vector.tensor_copy(ot[:, j % 4, :], ops_[:])
                if j % 4 == 3:
                    rcb = smallp.tile([128, 4], F32, tag="rcb", name="rcb")
                    nc.vector.reciprocal(rcb[:], ot[:, :, 128])
                    ne = nc.vector if prev[4] == 4 and j == 3 else oeng
                    ne.tensor_tensor(ot[:, :, 0:128], ot[:, :, 0:128],
                                     _bcast3(rcb[:], 128), op=OP.mult)
                    q0 = qbase + (j - 3) * 128
                    nc.sync.dma_start(
                        out=o_d[q0:q0 + 512, h, :].rearrange(
                            "(j p) d -> p j d", p=128),
                        in_=ot[:, :, 0:128])

            # Emission schedule: hooks[(h, qc, kt)] -> list of pre-work
            # thunks, spreading head h+1's preprocessing across head h's
            # main loop so the static per-engine schedule zippers instead
            # of head-of-line blocking.  K+Q chunks of head h are needed
            # from that head's qc0; V only from its qc1 (first PV).
            hds = {0: make_hd(0, split_first=split_first)}
            hooks = {}
            if kq_pos is None:
                kq_pos = [(1, 3), (1, 11), (1, 19), (1, 27), (2, 3), (2, 11),
                          (2, 19), (2, 27)]
            if v_pos is None:
                v_pos = [(3, 16), (3, 24)] + [(4, kt) for kt in
                                              (2, 7, 12, 17, 22, 27)]
            for h in range(1, H):
                hds[h] = make_hd(h)
                for c in range(n_ch):
                    qc, kt = kq_pos[c]
                    hooks.setdefault((h - 1, qc, kt), []).append(
                        (emit_kq, h, c))
                for c in range(n_ch):
                    qc, kt = v_pos[c]
                    hq, hqc = (h - 1, qc) if qc < n_qc else (h, qc - n_qc)
                    hooks.setdefault((hq, hqc, kt), []).append(
                        (emit_v, h, c))

            # Head-0 prologue: Q runs one chunk ahead of K (the first QK
            # matmul needs QT chunks 0 AND 1, but only KT chunk 0), V last
            # (first needed by PV at qc1).  The first K chunk is emitted as
            # two 256-token halves so exp(kt0) is not gated on a full
            # 512-token chain.  The first v_pool V chunks run on GpSimd:
            # the DVE prologue (K chains + copies + remaining V) would
            # otherwise finish after the first PV needs Vext.
            hd0 = hds[0]
            if split_first:
                emit_k(0, hd0, 0, j0=0, nj=2, dest=hd0["KT0h"][0],
                       ksdest=hd0["ks0h"][0], ksadest=hd0["ksA0h"][0])
                emit_q(0, hd0, 0)
                emit_q(0, hd0, 1)
                emit_k(0, hd0, 0, j0=2, nj=2, dest=hd0["KT0h"][1],
                       ksdest=hd0["ks0h"][1], ksadest=hd0["ksA0h"][1],
                       eng=nc.gpsimd)
            else:
                emit_k(0, hd0, 0)
                emit_q(0, hd0, 0)
                emit_q(0, hd0, 1)
            # Alternate head-0 prologue chains DVE/GpSimd so the serial
            # preprocessing phase runs on two engines concurrently. Late K
            # chunks and all V chunks are hooked into window (0,0)'s slot
            # stream instead of front-loading the prologue.
            def pick(pat, i):
                ch = pat[i % len(pat)]
                return nc.gpsimd if ch == "g" else None

            n_pre_k = n_ch - k0_hooks
            for c in range(1, n_ch):
                if c < n_pre_k:
                    emit_k(0, hd0, c, eng=pick(pre_k_eng, c - 1))
                if c + 1 < n_ch:
                    emit_q(0, hd0, c + 1)
            for i, c in enumerate(range(n_pre_k, n_ch)):
                hooks.setdefault((0, 0, 2 + 4 * i), []).append(
                    (functools.partial(emit_k, eng=pick(k0_eng, i)), 0, c))
            if v0_kts is None:
                for c in range(n_ch):
                    emit_v(0, hd0, c, eng=pick(v0_eng, c))
            else:
                for c in range(n_ch):
                    kt0 = v0_kts[c]
                    hooks.setdefault((0, 0, kt0), []).append(
                        (functools.partial(emit_v, eng=pick(v0_eng, c)), 0, c))

            prev = None       # completed (pts, vext, h, qbase, npv) block
            prev_ot = [None, None]

            def maybe_pv(kt):
                if prev is None:
                    return
                npv = prev[4]
                sp = n_kt // npv
                if kt % sp == 0 and kt // sp < npv:
                    emit_pv(prev, kt // sp)

            for h in range(H):
                hd = hds[h]
                for qc in range(n_qc):
                    last = (h == H - 1 and qc == n_qc - 1)
                    if not last:
                        pts = []
                        for kt in range(n_kt):
                            w, ksl, ksal = kt_weight(hd, kt)
                            dve = (dve_mod and kt % dve_mod == dve_off
                                   and (h, qc) not in dve_skip)
                            pt = ppool.tile([128, 1024], BF16, tag="pp")
                            if dve:
                                # DVE fast-exp tile: score halves go through
                                # the single-bank psD pool so DVE latency
                                # never gates the ACT/psS pipeline.
                                for half in range(2):
                                    spd = psD.tile([128, 512], F32,
                                                   tag="spd", name="spd")
                                    nc.tensor.matmul(
                                        spd[:], w,
                                        hd["QT"][2 * qc + half][:],
                                        start=True, stop=True)
                                    nc.vector.tensor_scalar(
                                        pt[:, half * 512:(half + 1) * 512]
                                        .bitcast(I16), spd[:], ksal,
                                        _SCHRAUD_B, op0=OP.mult, op1=OP.add)
                            else:
                                sps = psS.tile([128, 1024], F32, tag="sps")
                                for half in range(2):
                                    nc.tensor.matmul(
                                        sps[:, half * 512:(half + 1) * 512],
                                        w, hd["QT"][2 * qc + half][:],
                                        start=True, stop=True)
                                nc.scalar.activation(pt[:], sps[:], EXP,
                                                     bias=bias_t[:],
                                                     scale=ksl)
                            pts.append(pt)
                            for fn, hh, cc in hooks.get((h, qc, kt), ()):
                                fn(hh, hds[hh], cc)
                            maybe_pv(kt)
                        prev = (pts, hd["V"], hd["h"], qc * 1024, 8)
                    else:
                        # Final block: two 512-col halves so the closing PV
                        # chain (otherwise ~14us of pure tail) halves, with
                        # half A's PV overlapping half B's exps.
                        for half in range(2):
                            pts = []
                            for kt in range(n_kt):
                                w, ksl, ksal = kt_weight(hd, kt)
                                dve = (dve_mod and kt % dve_mod == dve_off
                                       and (h, qc) not in dve_skip)
                                pt = ppool.tile([128, 512], BF16, tag="pp",
                                                name="pth")
                                if dve:
                                    spd = psD.tile([128, 512], F32,
                                                   tag="spd", name="spd")
                                    nc.tensor.matmul(
                                        spd[:], w,
                                        hd["QT"][2 * qc + half][:],
                                        start=True, stop=True)
                                    nc.vector.tensor_scalar(
                                        pt[:].bitcast(I16), spd[:], ksal,
                                        _SCHRAUD_B, op0=OP.mult, op1=OP.add)
                                else:
                                    sps = psS.tile([128, 1024], F32,
                                                   tag="sps")
                                    nc.tensor.matmul(
                                        sps[:, 0:512], w,
                                        hd["QT"][2 * qc + half][:],
                                        start=True, stop=True)
                                    nc.scalar.activation(pt[:], sps[:, 0:512],
                                                         EXP, bias=bias_t[:],
                                                         scale=ksl)
                                pts.append(pt)
                                maybe_pv(kt)
                            prev = (pts, hd["V"], hd["h"],
                                    qc * 1024 + half * 512, 4)
            # Final drain: per-j normalize + 128-token stores so earlier
            # tiles stream out while PE finishes the later PV groups, and
            # the very last DMA is only a quarter-size transfer.
            pts_f, vext_f, h_f, qbase_f, npv_f = prev
            for j in range(npv_f):
                ops_ = psO.tile([128, 129], F32, tag="ops")
                for kt in range(n_kt):
                    nc.tensor.matmul(
                        ops_[:], pts_f[kt][:, j * 128:(j + 1) * 128],
                        vext_f[:, kt, 0:129],
                        start=(kt == 0), stop=(kt == n_kt - 1))
                otf = outp.tile([128, 129], F32, tag="ot", name="otf")
                nc.vector.tensor_copy(otf[:], ops_[:])
                rcf = smallp.tile([128, 1], F32, tag="rcb", name="rcf")
                nc.vector.reciprocal(rcf[:], otf[:, 128:129])
                nc.vector.tensor_scalar(otf[:, 0:128], otf[:, 0:128],
                                        rcf[:], None, op0=OP.mult)
                q0 = qbase_f + j * 128
                nc.sync.dma_start(out=o_d[q0:q0 + 128, h_f, :],
                                  in_=otf[:, 0:128])

    nc.compile()
    return nc


def get_nc(**kwargs):
    key = tuple(sorted(kwargs.items()))
    if key not in _NC_CACHE:
        _NC_CACHE[key] = _build_nc(**kwargs)
    return _NC_CACHE[key]


def kernel(q, k, v, _trace=False, _trace_cores=None, _nc_kwargs=None):
    """Full-input entry point: q/k/v [1, 4096, 16, 128] fp32 -> same shape."""
    assert q.shape == (1, _S, _H_TOTAL, _D), q.shape
    nc = get_nc(**(_nc_kwargs or {}))
    in_maps = []
    for c in range(_N_CORES):
        hs = slice(c * _H, (c + 1) * _H)
        in_maps.append({
            "q": np.ascontiguousarray(q[0, :, hs, :], dtype=np.float32),
            "k": np.ascontiguousarray(k[0, :, hs, :], dtype=np.float32),
            "v": np.ascontiguousarray(v[0, :, hs, :], dtype=np.float32),
        })
    # The axon-tunneled device occasionally reports a transient
    # NRT_EXEC_UNIT_UNRECOVERABLE on the first execution; a retry succeeds.
    last_err = None
    for attempt in range(3):
        try:
            res = run_bass_kernel_spmd(nc, in_maps,
                                       core_ids=list(range(_N_CORES)),
                                       trace=_trace, trace_cores=_trace_cores)
            break
        except Exception as e:  # noqa: BLE001
            last_err = e
            time.sleep(2.0 * (attempt + 1))
    else:
        raise last_err
    out = np.concatenate([res.results[c]["o"] for c in range(_N_CORES)],
                         axis=1)[None]
    out = np.ascontiguousarray(out, dtype=np.float32)
    if _trace:
        return out, res
    return out

